# revision 11
# baseline (speedup 1.0000x reference)
"""Cross-modal attention kernel for Trainium2, 8 NeuronCores.

Problem (nn_CrossModalAttention): B=2, N=2048, DIM=768, HEADS=12, HD=64.
  q/k/v = Linear(x{1,2}); attn blend: a1 = softmax((1-s)*q1k1 + s*q1k2),
  a2 = softmax((1-s)*q2k2 + s*q2k1); out = (a@v) @ Wo^T + bo.

Key algebraic folds (host side):
  - (1-s)*q1k1' + s*q1k2' = q1 @ ((1-s)k1 + s*k2)' and k is linear in x, so
    kb1 = ((1-s)x1 + s*x2) @ Wk^T + bk.  Two standard attentions remain.
  - softmax scale folded into Wq/bq.

Sharding: 8 cores = 2 (batch) x 2 (modality) x 2 (head halves of 6 heads).
Each core computes a partial output projection over its 6 heads; host sums
the two head-half partials and adds bo.

Device-side per core (all matmuls in float32r: full PE rate, ~1e-4 rounding):
  A: PE-transpose x_loc, xb_loc -> xT, xbT ([c, n] layout)
  B: projections qT = w_q^T.T @ xT (bias on DVE), kbT likewise from xbT,
     v native = xT.T @ w_v (+bias+ones column for the sum row)
  C: per head pair: scoresT = kbT_h.T-slices @ qT (row-packed 2 heads),
     exp on ACT (no max subtraction needed: |scores| <= ~8),
     oT65 = [v_h | 1].T @ expS accumulated over 16 key tiles (row 64 = sum),
     normalize via DVE reciprocal + ones-matmul partition broadcast.
  D: partial out = oT.T @ w_o slices, DMA out.
"""

import os
import sys
import time

for _p in ("/opt/trn_rl_repo", "/root/.axon_site/_ro/trn_rl_repo"):
    if os.path.isdir(_p) and _p not in sys.path:
        sys.path.insert(0, _p)

import numpy as np

import concourse.bass as bass
import concourse.tile as tile
from concourse import bacc, mybir
from concourse.bass_utils import run_bass_kernel_spmd
from concourse.masks import make_identity

F32 = mybir.dt.float32
F32R = mybir.dt.float32r
AF = mybir.ActivationFunctionType

# Problem constants
B = 2
NQ = 2048  # sequence length
C = 768  # model dim
HD = 64  # head dim
HL = 6  # heads per core (half of 12)
DL = HL * HD  # 384 local head dims
P = 128
NT = NQ // P  # 16 n tiles
CT = C // P  # 6 contraction tiles
DT = DL // P  # 3 local d tiles (= head pairs)
QH = 1024  # q half (PSUM budget)
SCALE = HD ** -0.5

_cache = {}


def _build_program():
    nc = bacc.Bacc("TRN2", target_bir_lowering=False, debug=False, num_devices=8)

    x_ext = nc.declare_dram_parameter("x_loc", [NQ, C], F32, isOutput=False)
    xb_ext = nc.declare_dram_parameter("xb_loc", [NQ, C], F32, isOutput=False)
    wq_ext = nc.declare_dram_parameter("w_q", [C, DL], F32R, isOutput=False)
    wk_ext = nc.declare_dram_parameter("w_k", [C, DL], F32R, isOutput=False)
    wv_ext = nc.declare_dram_parameter("w_v", [C, DL], F32R, isOutput=False)
    wo_ext = nc.declare_dram_parameter("w_o", [DL, C], F32R, isOutput=False)
    bq_ext = nc.declare_dram_parameter("b_q", [DL], F32, isOutput=False)
    bk_ext = nc.declare_dram_parameter("b_k", [DL], F32, isOutput=False)
    bv_ext = nc.declare_dram_parameter("b_v", [HL * (HD + 1)], F32, isOutput=False)
    part_ext = nc.declare_dram_parameter("part", [NQ, C], F32, isOutput=True)

    with tile.TileContext(nc) as tc:
        _trace(nc, tc, x_ext, xb_ext, wq_ext, wk_ext, wv_ext, wo_ext,
               bq_ext, bk_ext, bv_ext, part_ext)
    nc.compile()
    return nc


def _trace(nc, tc, x_ext, xb_ext, wq_ext, wk_ext, wv_ext, wo_ext,
           bq_ext, bk_ext, bv_ext, part_ext):
    from contextlib import ExitStack

    consts = tc.alloc_tile_pool(name="consts", bufs=1)
    qkv_pool = tc.alloc_tile_pool(name="qkv", bufs=1)
    attn_sb = tc.alloc_tile_pool(name="attn_sb", bufs=1)

    ident = consts.tile([P, P], F32)
    make_identity(nc, ident[:])
    bq_sb = consts.tile([P, DT], F32)
    nc.sync.dma_start(bq_sb[:], bq_ext[:].rearrange("(t p) -> p t", p=P))
    bk_sb = consts.tile([P, DT], F32)
    nc.sync.dma_start(bk_sb[:], bk_ext[:].rearrange("(t p) -> p t", p=P))
    bv_sb = consts.tile([P, HL * (HD + 1)], F32)
    nc.sync.dma_start(bv_sb[:], bv_ext[:].partition_broadcast(P))
    wo_sb = consts.tile([P, DT, C], F32R)
    nc.sync.dma_start(wo_sb[:], wo_ext[:].rearrange("(t p) c -> p t c", p=P))

    # persistent qkv results
    qT = qkv_pool.tile([P, DT, NQ], F32R)  # [d, n] per pair chunk
    kbT = qkv_pool.tile([P, DT, NQ], F32R)
    v65 = qkv_pool.tile([P, NT, HL * (HD + 1)], F32R)  # v + ones column per head
    oT = attn_sb.tile([P, DT, NQ], F32R)

    # ---------------- phase A/B: transpose + projections -----------------
    with ExitStack() as ab:
        wpool = ab.enter_context(tc.tile_pool(name="w_qkv", bufs=1))
        xT_pool = ab.enter_context(tc.tile_pool(name="xT", bufs=1))
        xnat_pool = ab.enter_context(tc.tile_pool(name="x_nat", bufs=3))
        tp_ps = ab.enter_context(tc.tile_pool(name="tp_ps", bufs=2, space="PSUM"))
        pj_ps = ab.enter_context(tc.tile_pool(name="pj_ps", bufs=2, space="PSUM"))

        wq_sb = wpool.tile([P, CT, DL], F32R, tag="wq")
        nc.sync.dma_start(wq_sb[:], wq_ext[:].rearrange("(t p) d -> p t d", p=P))
        wk_sb = wpool.tile([P, CT, DL], F32R, tag="wk")
        nc.sync.dma_start(wk_sb[:], wk_ext[:].rearrange("(t p) d -> p t d", p=P))
        wv_sb = wpool.tile([P, CT, DL], F32R, tag="wv")
        nc.sync.dma_start(wv_sb[:], wv_ext[:].rearrange("(t p) d -> p t d", p=P))

        def transpose_in(src_ext, xT_tile):
            # src [n, c] -> xT [c-part, ct, n]
            for nt in range(NT):
                x_nat = xnat_pool.tile([P, C], F32, tag="xnat")
                nc.sync.dma_start(x_nat[:], src_ext[nt * P:(nt + 1) * P, :])
                ps = tp_ps.tile([P, CT * P], F32, tag="tp")
                for ct in range(CT):
                    nc.tensor.transpose(
                        ps[:, ct * P:(ct + 1) * P],
                        x_nat[:, ct * P:(ct + 1) * P],
                        ident[:],
                    )
                nc.vector.tensor_copy(
                    xT_tile[:, :, nt * P:(nt + 1) * P],
                    ps[:].rearrange("p (ct x) -> p ct x", x=P),
                )

        def proj_T(w_sb, b_sb, xT_tile, out_tile):
            # out[d-part, dt, n] = w.T @ xT  (+ per-partition bias)
            for dt in range(DT):
                for nq in range(NQ // 512):
                    ps = pj_ps.tile([P, 512], F32, tag="pj")
                    for ct in range(CT):
                        nc.tensor.matmul(
                            ps[:],
                            wq_lhsT(w_sb, ct, dt),
                            xT_tile[:, ct, nq * 512:(nq + 1) * 512],
                            start=(ct == 0),
                            stop=(ct == CT - 1),
                        )
                    nc.vector.tensor_scalar_add(
                        out_tile[:, dt, nq * 512:(nq + 1) * 512],
                        ps[:],
                        b_sb[:, dt:dt + 1],
                    )

        def wq_lhsT(w_sb, ct, dt):
            return w_sb[:, ct, dt * P:(dt + 1) * P]

        # x_loc: transposes, then v (native) and qT projections
        xT = xT_pool.tile([P, CT, NQ], F32R, tag="xT")
        transpose_in(x_ext, xT)

        # v native: [n-part, d] per n tile, plus bias and ones column
        for nt in range(NT):
            ps = pj_ps.tile([P, DL], F32, tag="pv")
            for ct in range(CT):
                nc.tensor.matmul(
                    ps[:],
                    xT[:, ct, nt * P:(nt + 1) * P],
                    wv_sb[:, ct, :],
                    start=(ct == 0),
                    stop=(ct == CT - 1),
                )
            vv = v65[:, nt, :].rearrange("p (h x) -> p h x", x=HD + 1)
            bvv = bv_sb[:].rearrange("p (h x) -> p h x", x=HD + 1)
            nc.vector.tensor_add(
                vv[:, :, 0:HD],
                ps[:].rearrange("p (h x) -> p h x", x=HD),
                bvv[:, :, 0:HD],
            )
            nc.vector.tensor_copy(vv[:, :, HD:HD + 1], bvv[:, :, HD:HD + 1])

        proj_T(wq_sb, bq_sb, xT, qT)

        # xb_loc: transposes reuse the xT slot, then kbT projection
        xbT = xT_pool.tile([P, CT, NQ], F32R, tag="xT")
        transpose_in(xb_ext, xbT)

        for dt in range(DT):
            for nq in range(NQ // 512):
                ps = pj_ps.tile([P, 512], F32, tag="pj")
                for ct in range(CT):
                    nc.tensor.matmul(
                        ps[:],
                        wq_lhsT(wk_sb, ct, dt),
                        xbT[:, ct, nq * 512:(nq + 1) * 512],
                        start=(ct == 0),
                        stop=(ct == CT - 1),
                    )
                nc.vector.tensor_scalar_add(
                    kbT[:, dt, nq * 512:(nq + 1) * 512],
                    ps[:],
                    bk_sb[:, dt:dt + 1],
                )

    # ---------------- phase C: attention ---------------------------------
    with ExitStack() as cd:
        sc_ps = cd.enter_context(tc.tile_pool(name="sc_ps", bufs=2, space="PSUM"))
        av_ps = cd.enter_context(tc.tile_pool(name="av_ps", bufs=1, space="PSUM"))
        expp = cd.enter_context(tc.tile_pool(name="expp", bufs=3))
        nrm = cd.enter_context(tc.tile_pool(name="nrm", bufs=2))

        for pair in range(DT):
            for qh in range(NQ // QH):
                qs = qh * QH
                avs = [av_ps.tile([HD + 1, QH], F32, tag=f"av{h}", name=f"av{h}")
                       for h in range(2)]
                for krt in range(NT):
                    sps = [sc_ps.tile([P, QH], F32, tag="sc", name=f"sc{h}")
                           for h in range(2)]
                    for h in range(2):
                        lo = h * HD
                        for cq in range(QH // 512):
                            nc.tensor.matmul(
                                sps[h][:, cq * 512:(cq + 1) * 512],
                                kbT[lo:lo + HD, pair, krt * P:(krt + 1) * P],
                                qT[lo:lo + HD, pair, qs + cq * 512:qs + (cq + 1) * 512],
                                start=True,
                                stop=True,
                            )
                    for h in range(2):
                        es = expp.tile([P, QH], F32R, tag="expS")
                        nc.scalar.activation(es[:], sps[h][:], AF.Exp)
                        for cq in range(QH // 512):
                            nc.tensor.matmul(
                                avs[h][:, cq * 512:(cq + 1) * 512],
                                v65[:, krt, (pair * 2 + h) * (HD + 1):
                                    (pair * 2 + h + 1) * (HD + 1)],
                                es[:, cq * 512:(cq + 1) * 512],
                                start=(krt == 0),
                                stop=(krt == NT - 1),
                            )
                # normalize: oT = av[0:64] * (1 / av[64]) broadcast over partitions
                for h in range(2):
                    rec = nrm.tile([1, QH], F32, tag="rec")
                    nc.vector.reciprocal(rec[:], avs[h][HD:HD + 1, :])
                    bc = nrm.tile([HD, QH], F32, tag="bc")
                    nc.gpsimd.partition_broadcast(bc[:], rec[:])
                    nc.vector.tensor_mul(
                        oT[h * HD:(h + 1) * HD, pair, qs:qs + QH],
                        avs[h][0:HD, :],
                        bc[:],
                    )

    # ---------------- phase D: output projection --------------------------
    with ExitStack() as pd:
        o_ps = pd.enter_context(tc.tile_pool(name="o_ps", bufs=2, space="PSUM"))
        out_pool = pd.enter_context(tc.tile_pool(name="outp", bufs=3))
        for nt in range(NT):
            ps = o_ps.tile([P, C], F32, tag="po")
            for half in range(2):
                w = 512 if half == 0 else C - 512
                for dt in range(DT):
                    nc.tensor.matmul(
                        ps[:, half * 512:half * 512 + w],
                        oT[:, dt, nt * P:(nt + 1) * P],
                        wo_sb[:, dt, half * 512:half * 512 + w],
                        start=(dt == 0),
                        stop=(dt == DT - 1),
                    )
            osb = out_pool.tile([P, C], F32, tag="osb")
            nc.vector.tensor_copy(osb[:], ps[:])
            nc.sync.dma_start(part_ext[nt * P:(nt + 1) * P, :], osb[:])

    attn_sb.release()
    qkv_pool.release()
    consts.release()


def _prep_in_maps(x1, x2, Wq, bq, Wk, bk, Wv, bv, Wo, bo, cross_scale):
    s = float(np.asarray(cross_scale).reshape(-1)[0])
    xb1 = ((1.0 - s) * x1 + s * x2).astype(np.float32)
    xb2 = ((1.0 - s) * x2 + s * x1).astype(np.float32)
    wq_s = (SCALE * Wq).astype(np.float32)
    bq_s = (SCALE * bq).astype(np.float32)
    xs = (x1, x2)
    xbs = (xb1, xb2)
    in_maps = []
    for core in range(8):
        b, mod, half = core >> 2, (core >> 1) & 1, core & 1
        hs = slice(half * DL, (half + 1) * DL)
        in_maps.append({
            "x_loc": np.ascontiguousarray(xs[mod][b]),
            "xb_loc": np.ascontiguousarray(xbs[mod][b]),
            "w_q": np.ascontiguousarray(wq_s[hs, :].T),
            "w_k": np.ascontiguousarray(Wk[hs, :].T),
            "w_v": np.ascontiguousarray(Wv[hs, :].T),
            "w_o": np.ascontiguousarray(Wo[:, hs].T),
            "b_q": np.ascontiguousarray(bq_s[hs]),
            "b_k": np.ascontiguousarray(bk[hs]),
            "b_v": np.ascontiguousarray(
                np.concatenate([bv[hs].reshape(HL, HD),
                                np.ones((HL, 1), np.float32)], axis=1).reshape(-1)),
        })
    return in_maps


def kernel(x1, x2, Wq, bq, Wk, bk, Wv, bv, Wo, bo, cross_scale, _trace_opts=None):
    args = [np.asarray(a, dtype=np.float32) for a in
            (x1, x2, Wq, bq, Wk, bk, Wv, bv, Wo, bo, cross_scale)]
    x1, x2, Wq, bq, Wk, bk, Wv, bv, Wo, bo, cross_scale = args

    if "nc" not in _cache:
        _cache["nc"] = _build_program()
    nc = _cache["nc"]

    in_maps = _prep_in_maps(x1, x2, Wq, bq, Wk, bk, Wv, bv, Wo, bo, cross_scale)
    res = run_bass_kernel_spmd(nc, in_maps, list(range(8)), **(_trace_opts or {}))
    _cache["last_results"] = res

    out1 = np.empty((B, NQ, C), np.float32)
    out2 = np.empty((B, NQ, C), np.float32)
    outs = (out1, out2)
    for b in range(B):
        for mod in range(2):
            core0 = (b << 2) | (mod << 1)
            outs[mod][b] = (res.results[core0]["part"]
                            + res.results[core0 + 1]["part"] + bo)
    return out1, out2
